# revision 1
# baseline (speedup 1.0000x reference)
"""3-layer GCN encoder on 8 Trainium2 NeuronCores (Bass/Tile).

Sharding: 1D node partition (contiguous ranges) across 8 cores.

Math: with dis = deg^-1/2, norm_e = dis[src]*dis[dst] and the self-loop
coefficient 1/deg = dis*dis, so

    out_i = dis_i * ( sum_{e: dst=i} (z*dis)[src_e]  +  (z*dis)_i ) + b

Pipelined schedule: each layer's aggregation (phase B) is emitted
group-by-group, interleaved with the NEXT layer's z matmuls (phase A)
and a 4-way chunked AllGather, so the collective and both phases
overlap. h_full/ag_in are double-buffered across layers to break the
WAR hazard between one layer's gathers and the next layer's AllGather.
Inter-layer activations stay in SBUF (transposed SBUF->SBUF for the
next matmul); no DRAM round trip.

Aggregation per core: edges binned by (dst-tile x src-chunk), bulk
dma_gather of source rows, 0/1 one-hot S matrices built on VectorE
(one is_equal per gather call), segment sums via TensorE matmuls
accumulating in PSUM, self term via an identity matmul, dis_i folded
into the ReLU's per-partition scale on ScalarE. LayerNorm (layers 1,2)
uses the var = E[x^2] - mu^2 form so only 3 full-width ScalarE ops run
per tile.
"""
import math

import numpy as np
import ml_dtypes

BF16 = ml_dtypes.bfloat16

# problem constants (hardcoded per contract)
N = 100000
E = 1600000
IN_DIM = 512
F = 256
LN_EPS = 1e-5
NCORES = 8
P = 128
BLOCKS = 4          # gather-table blocks == AllGather chunks


# ---------------------------------------------------------------- host side


def _preprocess(edge_index, n_pad, npc, nt, tt):
    """Bin edges by (dst-core, dst-tile, src-chunk); pack chunk metadata.

    The AllGather is chunked: chunk c carries within-core rows
    [c*S, (c+1)*S) from every core, landing at h_full rows
    [c*8S + r*S + (i - c*S)].  Gather block c == AllGather chunk c.

    Returns (kb, per-core dict of packed arrays). All shapes identical
    across cores (SPMD requirement).
    """
    src = np.ascontiguousarray(edge_index[0]).astype(np.int64)
    dst = np.ascontiguousarray(edge_index[1]).astype(np.int64)
    S = npc // BLOCKS                   # chunk rows per core

    deg = 1.0 + np.bincount(dst, minlength=n_pad).astype(np.float64)
    dis = (1.0 / np.sqrt(deg)).astype(np.float32)

    core = dst // npc
    parts = []
    kmax = 1
    for r in range(NCORES):
        m = core == r
        s_r, d_r = src[m], dst[m]
        tid = (d_r - r * npc) >> 7
        blk = (s_r % npc) // S          # chunk of the source node
        rel = (s_r // npc) * S + (s_r % npc) - blk * S
        key = tid * BLOCKS + blk
        o = np.argsort(key, kind="stable")
        rel_s, d_s, key_s = rel[o], d_r[o], key[o]
        cnt = np.bincount(key_s, minlength=nt * BLOCKS)
        starts = np.concatenate([[0], np.cumsum(cnt)])
        pos = np.arange(len(key_s)) - starts[key_s]
        kmax = max(kmax, int(math.ceil(cnt.max() / P)))
        parts.append((rel_s, d_s, key_s, pos))

    kb = kmax
    ng = nt // tt

    out = []
    for r in range(NCORES):
        rel_s, d_s, key_s, pos = parts[r]
        # slot arrays [nt*BLOCKS, kb*128]; pads: idx 0 (re-gathers block
        # row 0), dstloc -1 (never matches the iota compare)
        si = np.zeros((nt * BLOCKS, kb * P), np.int64)
        dl = np.full((nt * BLOCKS, kb * P), -1.0, np.float32)
        si[key_s, pos] = rel_s
        dl[key_s, pos] = (d_s % P).astype(np.float32)
        assert rel_s.min() >= 0 and rel_s.max() < S * NCORES <= 32768

        # dstloc device layout: [128, BLOCKS*nt*kb], col = b*(nt*kb)+t*kb+j
        # so the (g, b) slice for the merged is_equal is contiguous.
        dstloc = np.ascontiguousarray(
            dl.reshape(nt, BLOCKS, kb, P)
            .transpose(3, 1, 0, 2)
            .reshape(P, BLOCKS * nt * kb)
        )

        # gather idx stream: per (group, block) call, flat order
        # i = (tl*kb + j)*128 + p
        si4 = si.reshape(nt, BLOCKS, kb * P)
        cols = []
        for g in range(ng):
            for b in range(BLOCKS):
                flat = si4[g * tt:(g + 1) * tt, b, :].reshape(-1)
                cols.append(flat.reshape(-1, 16).T.astype(np.int16))
        idxs = np.tile(np.concatenate(cols, axis=1), (NCORES, 1))

        dv = np.ascontiguousarray(
            dis[r * npc:(r + 1) * npc].astype(np.float32).reshape(nt, P).T
        )
        out.append(
            dict(
                idxs=np.ascontiguousarray(idxs),
                dstloc=dstloc.astype(BF16),
                dis=dv,
            )
        )
    return kb, out


# ---------------------------------------------------------------- device side


def _build_program(npc, nt, tt, kb, in_dim, f, ncores, debug=False):
    """Build the 3-layer GCN SPMD program. Returns compiled Bacc."""
    from concourse import bass, mybir, tile, bacc
    from concourse.masks import make_identity
    from concourse.library_config import mlp

    bf = mybir.dt.bfloat16
    f32 = mybir.dt.float32
    AF = mybir.ActivationFunctionType
    Alu = mybir.AluOpType

    kt = BLOCKS * kb
    ng = nt // tt
    n_pad = npc * ncores
    S = npc // BLOCKS                    # AllGather chunk rows per core
    CS = S * ncores                      # gather-table rows per block
    nidx_call = tt * kb * P
    l16 = nidx_call // 16
    idx_cols = ng * BLOCKS * l16
    nkb = [in_dim // P, f // P, f // P]  # contraction blocks per layer

    # 32KB descriptor carveout: the default 16KB ring fits only one
    # 4480-idx gather call per SWDGE queue, so every prep blocked in
    # await_space until the previous same-queue call drained.
    nc = bacc.Bacc("TRN2", target_bir_lowering=False, debug=False,
                   enable_asserts=True, num_devices=ncores,
                   num_swdge_queues=4, dynamic_dma_scratch_size=32768)

    # ---- I/O
    # x pre-packed per tile: row (t*128+p), col (kbi*128+q) holds
    # x[t*128+q, kbi*128+p]  (lhsT layout, one contiguous DMA per tile)
    x_in = nc.dram_tensor("x", [npc, in_dim], bf, kind="ExternalInput")
    w_in = [
        nc.dram_tensor("w1", [in_dim, f], bf, kind="ExternalInput"),
        nc.dram_tensor("w2", [f, f], bf, kind="ExternalInput"),
        nc.dram_tensor("w3", [f, f], bf, kind="ExternalInput"),
    ]
    idxs_in = nc.dram_tensor("idxs", [P, idx_cols], mybir.dt.int16,
                             kind="ExternalInput")
    dstloc_in = nc.dram_tensor("dstloc", [P, BLOCKS * nt * kb], bf,
                               kind="ExternalInput")
    dis_in = nc.dram_tensor("dis", [P, nt], f32, kind="ExternalInput")
    y_out = nc.dram_tensor("y", [npc, f], f32, kind="ExternalOutput")
    if debug:
        dbg_z = nc.dram_tensor("dbg_z", [npc, f], bf, kind="ExternalOutput")
        dbg_h = nc.dram_tensor("dbg_h", [npc * ncores, f], bf,
                               kind="ExternalOutput")
        dbg_vr = nc.dram_tensor("dbg_vr", [npc, f], f32,
                                kind="ExternalOutput")
        dbg_y = nc.dram_tensor("dbg_y", [npc, f], bf, kind="ExternalOutput")
        dbg_xt = nc.dram_tensor("dbg_xt", [npc, f], bf,
                                kind="ExternalOutput")

    # ---- internal DRAM (double-buffered across layers)
    ag_in = [nc.dram_tensor(f"ag_in{i}", [npc, f], bf, kind="Internal")
             for i in range(2)]
    h_full = [nc.dram_tensor(f"h_full{i}", [n_pad, f], bf, kind="Internal",
                             addr_space="Shared") for i in range(2)]

    with tile.TileContext(nc) as tc:
        with tc.tile_pool(name="consts", bufs=1) as cpool, \
             tc.tile_pool(name="xload", bufs=3) as xpool, \
             tc.tile_pool(name="work", bufs=3) as wpool, \
             tc.tile_pool(name="vrp", bufs=9) as vrpool, \
             tc.tile_pool(name="ybuf", bufs=3) as ypool, \
             tc.tile_pool(name="xtp", bufs=3) as xtpool, \
             tc.tile_pool(name="zdl", bufs=6) as zpool, \
             tc.tile_pool(name="gather", bufs=5) as gpool, \
             tc.tile_pool(name="smat", bufs=3) as spool2, \
             tc.tile_pool(name="stats", bufs=8) as spool, \
             tc.tile_pool(name="psA", bufs=1, space="PSUM") as psA, \
             tc.tile_pool(name="psB", bufs=7, space="PSUM") as psB:

            # iota / identity first (gpsimd base ops), then the mlp library
            iota_sb = cpool.tile([P, tt * kb * P], bf, tag="iota")
            nc.gpsimd.iota(iota_sb[:], pattern=[[0, tt * kb], [1, P]],
                           base=0, channel_multiplier=0,
                           allow_small_or_imprecise_dtypes=True)
            ident_sb = cpool.tile([P, P], bf, tag="ident")
            make_identity(nc, ident_sb[:])
            eps_sb = cpool.tile([P, 1], f32, tag="eps")
            nc.gpsimd.memset(eps_sb[:], LN_EPS)
            nc.gpsimd.load_library(mlp)

            # ---- persistent constants in SBUF
            idxs_sb = cpool.tile([P, idx_cols], mybir.dt.int16, tag="idxs")
            nc.sync.dma_start(idxs_sb[:], idxs_in[:])
            dstloc_sb = cpool.tile([P, BLOCKS * nt * kb], bf, tag="dstloc")
            nc.sync.dma_start(dstloc_sb[:], dstloc_in[:])
            dis_sb = cpool.tile([P, nt], f32, tag="dis")
            nc.sync.dma_start(dis_sb[:], dis_in[:])

            w_sb = []
            for l, w in enumerate(w_in):
                kin = w.shape[0]
                blocks = []
                for b in range(kin // P):
                    t = cpool.tile([P, f], bf, tag=f"w{l}_{b}")
                    nc.sync.dma_start(t[:], w[b * P:(b + 1) * P, :])
                    blocks.append(t)
                w_sb.append(blocks)

            # AG chunk c is triggered once phase-A tiles covering rows
            # [c*S, (c+1)*S) are written; tile boundary (0-based, incl):
            ag_tile = [int(math.ceil((c + 1) * S / P)) - 1
                       for c in range(BLOCKS)]
            # ... which during phase B maps to a group boundary:
            ag_group = [int(math.ceil((t + 1) / tt)) - 1 for t in ag_tile]

            def emit_ag(l, c):
                """AllGather chunk c of layer l's table."""
                nc.gpsimd.collective_compute(
                    "AllGather",
                    Alu.bypass,
                    replica_groups=[list(range(ncores))],
                    ins=[ag_in[l % 2][c * S:(c + 1) * S, :]],
                    outs=[h_full[l % 2][c * CS:(c + 1) * CS, :]],
                )

            # One PSUM bank per layer for phase A, halves ping-ponged by
            # tile parity (PSUM pools are bank-granular; a [P, f] f32 tile
            # is half a bank).
            zp_pair = [None]

            def emit_a_tile(l, t, src_tile):
                """z_t = (in @ W_l) * dis for tile t -> ag_in[l%2].

                src_tile: SBUF lhsT tile [P, nkb[l]*P] (feature-major
                blocks along the free dim)."""
                if t == 0:
                    zp_pair[0] = psA.tile([P, 2, f], f32, tag="zpsum",
                                          name=f"zpsum_{l}")
                zp = zp_pair[0][:, t % 2, :]
                for kbi in range(nkb[l]):
                    nc.tensor.matmul(out=zp,
                                     lhsT=src_tile[:, kbi * P:(kbi + 1) * P],
                                     rhs=w_sb[l][kbi][:],
                                     start=(kbi == 0),
                                     stop=(kbi == nkb[l] - 1))
                zt = wpool.tile([P, f], bf, tag="zt")
                nc.scalar.activation(zt[:], zp, AF.Copy,
                                     scale=dis_sb[:, t:t + 1])
                nc.sync.dma_start(ag_in[l % 2][t * P:(t + 1) * P, :], zt[:])
                if debug and l == 0:
                    nc.sync.dma_start(dbg_z[t * P:(t + 1) * P, :], zt[:])

            # ================= phase A, layer 0 =================
            for t in range(nt):
                xt = xpool.tile([P, in_dim], bf, tag="xt")
                nc.sync.dma_start(xt[:], x_in[t * P:(t + 1) * P, :])
                emit_a_tile(0, t, xt)
                for c in range(BLOCKS):
                    if t == ag_tile[c]:
                        emit_ag(0, c)

            # ================= layers: B(l) fused with A(l+1) =================
            for l in range(3):
                par = l % 2
                for g in range(ng):
                    # ---- aggregation matmuls for this group's 7 tiles.
                    # One full PSUM bank per tile: interleaved accumulation
                    # chains sharing a bank corrupt each other.
                    tiles_b = [psB.tile([P, f], f32, tag="agg",
                                        name=f"agg_{l}_{g}_{i}")
                               for i in range(tt)]
                    aps = [tb[:] for tb in tiles_b]

                    def emit_iseq(eng, b):
                        s_gb = spool2.tile([P, tt * kb * P], bf, tag="st",
                                           name=f"st_{l}_{g}_{b}")
                        o = b * nt * kb + g * tt * kb
                        dl3 = dstloc_sb[:, o:o + tt * kb].to_broadcast(
                            [P, tt * kb, P])
                        s3 = s_gb[:].rearrange("p (k q) -> p k q", q=P)
                        i3 = iota_sb[:].rearrange("p (k q) -> p k q", q=P)
                        eng.tensor_tensor(s3, i3, dl3, op=Alu.is_equal)
                        return s_gb

                    # prefetch the self-term rows so the closing matmuls
                    # below never wait on the load
                    zdts = []
                    for tl in range(tt):
                        t = g * tt + tl
                        zdt = zpool.tile([P, f], bf, tag="zdl")
                        nc.scalar.dma_start(zdt[:],
                                            ag_in[par][t * P:(t + 1) * P, :])
                        zdts.append(zdt)

                    for b in range(BLOCKS):
                        gt = gpool.tile([P, tt * kb, f], bf, tag="gt")
                        co = (g * BLOCKS + b) * l16
                        nc.gpsimd.dma_gather(
                            gt[:], h_full[par][b * CS:(b + 1) * CS, :],
                            idxs_sb[:, co:co + l16],
                            nidx_call, nidx_call, f, single_packet=False,
                            queue_num=b)
                        s_gb = emit_iseq(nc.vector, b)
                        for tl in range(tt):
                            for j in range(kb):
                                ci = tl * kb + j
                                nc.tensor.matmul(
                                    out=aps[tl],
                                    lhsT=s_gb[:, ci * P:(ci + 1) * P],
                                    rhs=gt[:, ci, :],
                                    start=(b == 0 and j == 0), stop=False)

                    # ---- self term + ReLU drain (frees PSUM earliest)
                    vrs = []
                    for tl in range(tt):
                        t = g * tt + tl
                        nc.tensor.matmul(out=aps[tl], lhsT=ident_sb[:],
                                         rhs=zdts[tl][:], start=False,
                                         stop=True)
                        if l < 2:
                            vr = vrpool.tile([P, f], f32, tag="vr")
                            musum = spool.tile([P, 1], f32, tag="musum")
                            nc.scalar.activation(vr[:], aps[tl], AF.Relu,
                                                 scale=dis_sb[:, t:t + 1],
                                                 accum_out=musum[:])
                            vrs.append((vr, musum))
                            if debug and l == 0:
                                nc.sync.dma_start(
                                    dbg_vr[t * P:(t + 1) * P, :], vr[:])
                        else:
                            y_t = wpool.tile([P, f], f32, tag="yf")
                            nc.scalar.activation(y_t[:], aps[tl], AF.Relu,
                                                 scale=dis_sb[:, t:t + 1])
                            nc.sync.dma_start(y_out[t * P:(t + 1) * P, :],
                                              y_t[:])

                    # ---- LayerNorm (E[x^2]-mu^2 form) + next layer's A tile
                    if l < 2:
                        for tl in range(tt):
                            t = g * tt + tl
                            vr, musum = vrs[tl]
                            sq = wpool.tile([P, f], f32, tag="sq")
                            sqsum = spool.tile([P, 1], f32, tag="sqsum")
                            nc.scalar.activation(sq[:], vr[:], AF.Square,
                                                 accum_out=sqsum[:])
                            mu_n = spool.tile([P, 1], f32, tag="mun")
                            nc.scalar.activation(mu_n[:], musum[:], AF.Copy,
                                                 scale=-1.0 / f)
                            msq = spool.tile([P, 1], f32, tag="msq")
                            nc.scalar.activation(msq[:], sqsum[:], AF.Copy,
                                                 scale=1.0 / f)
                            mu2 = spool.tile([P, 1], f32, tag="mu2")
                            nc.vector.tensor_tensor(mu2[:], mu_n[:], mu_n[:],
                                                    op=Alu.mult)
                            var = spool.tile([P, 1], f32, tag="var")
                            nc.vector.tensor_tensor(var[:], msq[:], mu2[:],
                                                    op=Alu.subtract)
                            std = spool.tile([P, 1], f32, tag="std")
                            nc.scalar.activation(std[:], var[:], AF.Sqrt,
                                                 bias=eps_sb[:])
                            rs = spool.tile([P, 1], f32, tag="rs")
                            nc.vector.reciprocal(rs[:], std[:])
                            b2 = spool.tile([P, 1], f32, tag="b2")
                            nc.vector.tensor_tensor(b2[:], mu_n[:], rs[:],
                                                    op=Alu.mult)
                            y_t = ypool.tile([P, f], bf, tag="yt")
                            nc.scalar.activation(y_t[:], vr[:], AF.Identity,
                                                 scale=rs[:], bias=b2[:])
                            # transpose for next layer's lhsT (SBUF->SBUF)
                            xT = xtpool.tile([P, f], bf, tag="xT")
                            nc.sync.dma_start(
                                xT[:, 0:P], y_t[:, 0:P], transpose=True)
                            nc.sync.dma_start(
                                xT[:, P:2 * P], y_t[:, P:2 * P],
                                transpose=True)
                            if debug and l == 0:
                                nc.sync.dma_start(
                                    dbg_y[t * P:(t + 1) * P, :], y_t[:])
                                nc.sync.dma_start(
                                    dbg_xt[t * P:(t + 1) * P, :], xT[:])
                            emit_a_tile(l + 1, t, xT)
                        for c in range(BLOCKS):
                            if g == ag_group[c]:
                                emit_ag(l + 1, c)
                if debug and l == 0:
                    nc.sync.dma_start(dbg_h[:], h_full[0][:])

    nc.compile()
    return nc


# ---------------------------------------------------------------- entry point


def run_gcn(x, edge_index, Ws, n, e, in_dim, f, ncores=NCORES, tt=7,
            trace=False, debug=False):
    """Generic runner used by kernel() and by the mini test."""
    from concourse import bass_utils

    npc = int(math.ceil(n / (ncores * P))) * P
    n_pad = npc * ncores
    nt = npc // P
    while nt % tt:
        tt -= 1

    kb, meta = _preprocess(edge_index, n_pad, npc, nt, tt)

    x_pad = np.zeros((n_pad, in_dim), np.float32)
    x_pad[:n] = np.asarray(x, np.float32)
    # per-tile lhsT layout: row (t*128+p), col (kbi*128+q) = x[t*128+q,
    # kbi*128+p]
    nkb1 = in_dim // P
    x_bf = np.ascontiguousarray(
        x_pad.reshape(ncores, npc // P, P, nkb1, P)
        .transpose(0, 1, 4, 3, 2)
        .reshape(ncores, npc, in_dim)
    ).astype(BF16)
    w_bf = [np.asarray(w, np.float32).astype(BF16) for w in Ws]

    nc = _build_program(npc, nt, tt, kb, in_dim, f, ncores, debug=debug)

    in_maps = []
    for r in range(ncores):
        m = meta[r]
        in_maps.append({
            "x": x_bf[r],
            "w1": w_bf[0], "w2": w_bf[1], "w3": w_bf[2],
            "idxs": m["idxs"],
            "dstloc": m["dstloc"],
            "dis": m["dis"],
        })

    try:
        res = bass_utils.run_bass_kernel_spmd(
            nc, in_maps, core_ids=list(range(ncores)), trace=trace)
    except Exception:
        # transient device wedge (NRT_EXEC_UNIT_UNRECOVERABLE) — retry once
        res = bass_utils.run_bass_kernel_spmd(
            nc, in_maps, core_ids=list(range(ncores)), trace=trace)

    y = np.concatenate([res.results[r]["y"] for r in range(ncores)], axis=0)
    return y[:n], res


def kernel(x, edge_index, W1, b1, W2, b2, W3, b3, g1, be1, g2, be2):
    # b1..b3 are zeros and g/be are identity for this model; verified on host
    # (they fold away from the device program).
    assert not np.any(np.asarray(b1)) and not np.any(np.asarray(b2)) \
        and not np.any(np.asarray(b3))
    assert np.all(np.asarray(g1) == 1) and np.all(np.asarray(g2) == 1)
    assert not np.any(np.asarray(be1)) and not np.any(np.asarray(be2))

    y, _ = run_gcn(np.asarray(x), np.asarray(edge_index), [W1, W2, W3],
                   N, E, IN_DIM, F)
    return y

